# revision 16
# baseline (speedup 1.0000x reference)
"""Trainium2 Bass kernel for a 3-layer GCN encoder (PyG GCNConv x3 + global mean pool).

Strategy (8 NeuronCores):
  - Nodes sharded contiguously across cores (6250 each); edges (+self-loops)
    partitioned by destination, bucketed per 128-node dst block and split by
    source half (int16 gather-index limit), chunk-128 padded, sizes uniform
    across cores (SPMD program). Segments are laid out HALF-MAJOR so gather
    calls stream contiguously across block boundaries within a half: every
    dma_gather is a full 1024-idx window (no per-segment tail calls).
  - Symmetric normalization is folded on the host: ew'' = ew * dinv[src] *
    dinv[dst] (self-loop weight dinv[n]^2), so the device computes per layer
        agg[dst] = sum_e ew''_e * h[src_e]          (gathered bf16 rows)
        h'       = relu(agg @ W + b)
    with NO degree phase and NO per-layer dinv multiplies. The layer-0 table
    is x itself (bf16, replicated to every core's HBM by the host).
  - Gathers: dma_gather of 256B bf16 rows over 4 SWDGE queues into a rolling
    ring of static SBUF stage buffers. The descriptor ring is ucode-fixed at
    ~1024+16 entries per queue (larger dynamic_dma_scratch_size does NOT
    raise it), so calls are capped at 1024 idxs. Pads use idx=0 with ew=0
    (valid descriptor, zero contribution).
  - The weighted one-hot is built in TWO big-tile DVE tensor_tensor ops per
    (block,half) segment ((iota==slot), then *ew) with stride-0 broadcast
    APs — ~34x fewer DVE instructions than per-chunk tensor_scalar. The
    TensorEngine contracts gathered-rows x one-hot into PSUM (bf16 operands,
    f32 accumulate), seeds the bias with a K=1 ones x b matmul, applies W.
    Relu + PSUM->SBUF copies run on the otherwise-idle scalar engine.
  - h' (bf16) is written to the core's table shard and AllGathered between
    layers (bf16 halves collective bytes). Final global mean pool: per-block
    one-hot matmul accumulated in PSUM across all blocks; host sums the 8
    [64,128] partials and divides by graph counts.

Measured on 8xTRN2 (trace core 0): ~2.0 ms vs 5.64 ms for the f32
per-chunk-one-hot baseline; rel err 2.3e-3 (gate 2e-2).
Notes from tuning: 64-node dst blocks halve DVE/PE active time but REGRESS
wall time (~3.1 ms) — 2x the (block,half) iterations doubles cross-engine
semaphore traffic and the gather pipeline stalls; 1920/3968-idx gather calls
crash (ring cap); negative pad idxs require num_idxs_reg == #valid and
uniform tails; dropping even the lightest 10% of edges exceeds the error
budget.
"""

import os
import sys

import numpy as np

for _p in ("/opt/trn_rl_repo",):
    if _p not in sys.path and os.path.isdir(_p):
        sys.path.insert(0, _p)

import concourse.bass as bass
import concourse.bacc as bacc
import concourse.tile as tile
import concourse.mybir as mybir
from concourse import bass_utils
from concourse.alu_op_type import AluOpType

F32 = mybir.dt.float32
BF16 = mybir.dt.bfloat16
I16 = mybir.dt.int16
AF = mybir.ActivationFunctionType

N_QUEUES = int(os.environ.get("GCN_QUEUES", "4"))
CALL_CHUNKS = int(os.environ.get("GCN_CALLCH", "8"))
NBUF = int(os.environ.get("GCN_NBUF", "12"))   # stage-ring depth (calls)
NLOAD = 4  # const-load column slices (early gathers overlap the tail loads)


class Cfg:
    def __init__(self, n_nodes=50000, n_cores=8, d=128, n_graphs=64,
                 split=32768, blk=128):
        assert n_nodes % n_cores == 0
        self.n_nodes = n_nodes
        self.n_cores = n_cores
        self.d = d
        self.n_graphs = n_graphs
        self.split = split  # gather-table split point (int16 index limit)
        self.shard = n_nodes // n_cores
        self.blk = blk
        self.n_blk = (self.shard + blk - 1) // blk

    def slots(self, b):
        return min(self.blk, self.shard - b * self.blk)


def preprocess(cfg, edge_index, edge_weight):
    """Host-side: degrees/dinv, normalization folding, edge bucketing.
    Segments ordered half-major: [h=0: blocks 0..n-1][h=1: blocks 0..n-1]."""
    src = np.asarray(edge_index)[0].astype(np.int64)
    dst = np.asarray(edge_index)[1].astype(np.int64)
    ew = np.asarray(edge_weight).astype(np.float64)
    n = cfg.n_nodes
    loop = np.arange(n, dtype=np.int64)
    src = np.concatenate([src, loop])
    dst = np.concatenate([dst, loop])
    ew = np.concatenate([ew, np.ones(n, np.float64)])
    ne = src.shape[0]

    deg = np.bincount(dst, weights=ew, minlength=n)  # >= 1 (self-loop)
    dinv = 1.0 / np.sqrt(deg)
    ew = (ew * dinv[src] * dinv[dst]).astype(np.float32)

    core = dst // cfg.shard
    loc = dst - core * cfg.shard
    blkid = loc // cfg.blk
    slot = (loc - blkid * cfg.blk).astype(np.float32)
    half = (src >= cfg.split).astype(np.int64)
    key = core * (2 * cfg.n_blk) + half * cfg.n_blk + blkid

    order = np.argsort(key, kind="stable")
    nkeys = cfg.n_cores * cfg.n_blk * 2
    cnt = np.bincount(key, minlength=nkeys).reshape(cfg.n_cores, 2 * cfg.n_blk)
    # per-(half,block) chunk-padded sizes, shared across cores (SPMD program)
    seg = ((cnt.max(axis=0) + 127) // 128) * 128  # [2*n_blk], half-major
    seg_off = np.concatenate([[0], np.cumsum(seg)])  # [2*n_blk+1]
    ep = int(seg_off[-1])

    # position of each edge inside its padded per-core segment
    cnt_flat = np.bincount(key, minlength=nkeys)
    starts = np.concatenate([[0], np.cumsum(cnt_flat)])[:-1]
    sk = key[order]
    rank = np.arange(ne) - starts[sk]
    hb = sk % (2 * cfg.n_blk)
    pos = seg_off[hb] + rank
    core_s = sk // (2 * cfg.n_blk)

    # pads: idx=0 with slot/ew = 0 (valid descriptor, zero contribution)
    idx16 = np.zeros((cfg.n_cores, ep), np.int16)
    slotf = np.zeros((cfg.n_cores, ep), np.float32)
    ewf = np.zeros((cfg.n_cores, ep), np.float32)
    idx16[core_s, pos] = (src[order] - half[order] * cfg.split).astype(np.int16)
    slotf[core_s, pos] = slot[order]
    ewf[core_s, pos] = ew[order]

    # wrapped index layout: edge i -> [i%16, i//16], replicated to 128 partitions
    idxw = idx16.reshape(cfg.n_cores, ep // 16, 16).transpose(0, 2, 1)
    idxw = np.tile(idxw, (1, 8, 1)).copy()  # [cores, 128, ep//16]
    # chunk layout for the one-hot build: edge i -> [i%128, i//128], bf16
    import ml_dtypes
    bf16 = ml_dtypes.bfloat16
    slotw = np.ascontiguousarray(
        slotf.reshape(cfg.n_cores, ep // 128, 128).transpose(0, 2, 1)
    ).astype(bf16)
    eww = np.ascontiguousarray(
        ewf.reshape(cfg.n_cores, ep // 128, 128).transpose(0, 2, 1)
    ).astype(bf16)

    return dict(seg=seg, seg_off=seg_off, ep=ep,
                idxw=idxw, slotw=slotw, eww=eww)


def build_program(cfg, seg, seg_off, ep, trunc=""):
    """Build the SPMD Bass/Tile program. Trip counts depend only on seg/ep.

    trunc: debug knob - "G" gathers only, "OH" +one-hot, "L0"/"L1" stop
    after layer 0/1 (outputs garbage)."""
    n_blk, d, g64, blk = cfg.n_blk, cfg.d, cfg.n_graphs, cfg.blk
    nc = bacc.Bacc("TRN2", target_bir_lowering=False, debug=False,
                   enable_asserts=False, num_devices=cfg.n_cores,
                   num_swdge_queues=N_QUEUES)

    t0_in = nc.dram_tensor("t0", [cfg.n_nodes, d], BF16, kind="ExternalInput")
    idx_in = nc.dram_tensor("idxw", [128, ep // 16], I16, kind="ExternalInput")
    slot_in = nc.dram_tensor("slotw", [128, ep // 128], BF16, kind="ExternalInput")
    ew_in = nc.dram_tensor("eww", [128, ep // 128], BF16, kind="ExternalInput")
    pool_in = nc.dram_tensor("poolm", [blk, n_blk * g64], BF16, kind="ExternalInput")
    iota_in = nc.dram_tensor("iota", [128, blk], BF16, kind="ExternalInput")
    w_in = nc.dram_tensor("wmats", [3, d, d], BF16, kind="ExternalInput")
    b_in = nc.dram_tensor("biasb", [3, 1, d], BF16, kind="ExternalInput")
    out_t = nc.dram_tensor("pool_out", [g64, d], F32, kind="ExternalOutput")

    g_loc = [None] + [nc.dram_tensor(f"g_loc{k}", [cfg.shard, d], BF16,
                                     kind="Internal") for k in (1, 2)]
    g_full = [None] + [nc.dram_tensor(f"g_full{k}", [cfg.n_nodes, d], BF16,
                                      kind="Internal", addr_space="Shared")
                       for k in (1, 2)]
    rg = [list(range(cfg.n_cores))]

    n_layers = {"G": 1, "OH": 1, "L0": 1, "L1": 2}.get(trunc, 3)
    stop_at = {"G": 1, "OH": 2}.get(trunc, 99)

    # chunk bookkeeping (chunks = 128-edge groups, globally half-major)
    nch = [[int(seg[h * n_blk + b]) // 128 for b in range(n_blk)]
           for h in (0, 1)]
    cbase = [[int(seg_off[h * n_blk + b]) // 128 for b in range(n_blk)]
             for h in (0, 1)]
    half_c0 = [0, int(seg_off[n_blk]) // 128]          # half's first chunk
    half_c1 = [int(seg_off[n_blk]) // 128, ep // 128]  # half's end

    with tile.TileContext(nc) as tc:
        with tc.tile_pool(name="const", bufs=1) as cp:
            iota_sb = cp.tile([128, blk], BF16, tag="iota")
            nc.sync.dma_start(iota_sb[:, :], iota_in.ap())
            ones_sb = cp.tile([1, blk], BF16, tag="ones")
            nc.vector.memset(ones_sb[:, :], 1.0)
            w_sb = []
            b_sb = []
            for k in range(3):
                wt = cp.tile([d, d], BF16, tag=f"w{k}", name=f"w{k}")
                nc.sync.dma_start(wt[:, :], w_in.ap()[k, :, :])
                w_sb.append(wt)
                bt = cp.tile([1, d], BF16, tag=f"b{k}", name=f"b{k}")
                nc.sync.dma_start(bt[:, :], b_in.ap()[k, :, :])
                b_sb.append(bt)
            poolm_sb = cp.tile([blk, n_blk * g64], BF16, tag="poolm")
            nc.sync.dma_start(poolm_sb[:, :], pool_in.ap())
            # sliced const loads so early gathers don't wait on one huge DMA
            idx_sb = cp.tile([128, ep // 16], I16, tag="idx")
            slot_sb = cp.tile([128, ep // 128], BF16, tag="slot")
            ew_sb = cp.tile([128, ep // 128], BF16, tag="ew")
            w16 = ep // 16 // NLOAD
            w128 = ep // 128 // NLOAD
            for i in range(NLOAD):
                a16 = i * w16
                b16 = (i + 1) * w16 if i < NLOAD - 1 else ep // 16
                a12 = i * w128
                b12 = (i + 1) * w128 if i < NLOAD - 1 else ep // 128
                nc.sync.dma_start(idx_sb[:, a16:b16], idx_in.ap()[:, a16:b16])
                nc.sync.dma_start(slot_sb[:, a12:b12], slot_in.ap()[:, a12:b12])
                nc.sync.dma_start(ew_sb[:, a12:b12], ew_in.ap()[:, a12:b12])
            # rolling gather-stage ring: one slot per dma_gather call
            ring = [cp.tile([128, CALL_CHUNKS, d], BF16, tag=f"ring{i}",
                            name=f"ring{i}") for i in range(NBUF)]

            with tc.tile_pool(name="aggp", bufs=2, space="PSUM") as psA, \
                 tc.tile_pool(name="outp", bufs=2, space="PSUM") as psB, \
                 tc.tile_pool(name="poolp", bufs=1, space="PSUM") as psC, \
                 tc.tile_pool(name="ohp", bufs=4) as ohp, \
                 tc.tile_pool(name="workp", bufs=3) as wp:
                pool_acc = psC.tile([g64, d], F32, tag="pacc")
                state = {"seq": 0}
                for k in range(n_layers):
                    gsrc = t0_in.ap() if k == 0 else g_full[k].ap()
                    bases = (gsrc[0:cfg.split, :],
                             gsrc[cfg.split:cfg.n_nodes, :])
                    issued = [0, 0]        # calls issued per half
                    slot_of = [{}, {}]     # local call index -> ring slot

                    def ensure(h, gc_end):
                        """Issue calls for half h until global chunk gc_end
                        (exclusive) is covered."""
                        while (half_c0[h] + issued[h] * CALL_CHUNKS) < gc_end:
                            j = issued[h]
                            c0 = half_c0[h] + j * CALL_CHUNKS
                            c1 = min(c0 + CALL_CHUNKS, half_c1[h])
                            nidx = (c1 - c0) * 128
                            rs = state["seq"] % NBUF
                            state["seq"] += 1
                            nc.gpsimd.dma_gather(
                                ring[rs][:, 0:c1 - c0, :], bases[h],
                                idx_sb[:, c0 * 8:c0 * 8 + nidx // 16],
                                nidx, nidx, d,
                                queue_num=state["seq"] % N_QUEUES)
                            slot_of[h][j] = rs
                            issued[h] += 1

                    for b in range(n_blk):
                        s = cfg.slots(b)
                        ohg = {}
                        for h in (0, 1):
                            if nch[h][b] == 0:
                                continue
                            gs = cbase[h][b]
                            ge = gs + nch[h][b]
                            ensure(h, ge)
                            if stop_at < 2:
                                continue
                            n_ch = nch[h][b]
                            oh = ohp.tile([128, n_ch, blk], BF16, tag=f"oh{h}",
                                          name=f"oh{k}_{b}_{h}")
                            iota_b = (iota_sb[:, :].unsqueeze(1)
                                      .broadcast_to([128, n_ch, blk]))
                            slot_b = (slot_sb[:, gs:ge].unsqueeze(2)
                                      .broadcast_to([128, n_ch, blk]))
                            ew_b = (ew_sb[:, gs:ge].unsqueeze(2)
                                    .broadcast_to([128, n_ch, blk]))
                            nc.vector.tensor_tensor(
                                oh[:, :, :], iota_b, slot_b,
                                AluOpType.is_equal)
                            nc.vector.tensor_tensor(
                                oh[:, :, :], oh[:, :, :], ew_b,
                                AluOpType.mult)
                            ohg[h] = oh
                        if stop_at < 3:
                            continue
                        pagg = psA.tile([d, blk], F32, tag="agg",
                                        name=f"agg{k}_{b}")
                        ncols = nch[0][b] + nch[1][b]
                        j = 0
                        for h in (0, 1):
                            for i in range(nch[h][b]):
                                gc = cbase[h][b] + i
                                lc = gc - half_c0[h]
                                st = ring[slot_of[h][lc // CALL_CHUNKS]]
                                nc.tensor.matmul(
                                    pagg[:, :s],
                                    st[:, lc % CALL_CHUNKS, :],
                                    ohg[h][:, i, :s],
                                    start=(j == 0), stop=(j == ncols - 1))
                                j += 1
                        aggT = wp.tile([d, blk], BF16, tag="aggT",
                                       name=f"aggT{k}_{b}")
                        nc.scalar.copy(aggT[:, :s], pagg[:, :s])
                        pout = psB.tile([blk, d], F32, tag="out",
                                        name=f"out{k}_{b}")
                        nc.tensor.matmul(pout[:s, :], ones_sb[:, :s],
                                         b_sb[k][:, :], start=True, stop=False)
                        nc.tensor.matmul(pout[:s, :], aggT[:, :s], w_sb[k][:, :],
                                         start=False, stop=True)
                        if k < 2:
                            ht = wp.tile([blk, d], BF16, tag="ht",
                                         name=f"ht{k}_{b}")
                            nc.scalar.activation(ht[:s, :], pout[:s, :], AF.Relu)
                            nc.sync.dma_start(
                                g_loc[k + 1].ap()[b * blk:b * blk + s, :],
                                ht[:s, :])
                        else:
                            t2 = wp.tile([blk, d], BF16, tag="t2",
                                         name=f"t2_{b}")
                            nc.scalar.copy(t2[:s, :], pout[:s, :])
                            nc.tensor.matmul(
                                pool_acc[:, :],
                                poolm_sb[:s, b * g64:(b + 1) * g64],
                                t2[:s, :], start=(b == 0),
                                stop=(b == n_blk - 1))
                    if k < 2 and stop_at >= 3:
                        nc.gpsimd.collective_compute(
                            "AllGather", AluOpType.bypass, replica_groups=rg,
                            ins=[g_loc[k + 1].ap()], outs=[g_full[k + 1].ap()])

                if n_layers == 3 and stop_at >= 3:
                    pf = wp.tile([g64, d], F32, tag="pf", name="poolf")
                    nc.scalar.copy(pf[:, :], pool_acc[:, :])
                    nc.sync.dma_start(out_t.ap(), pf[:, :])

    nc.compile()
    return nc


def make_in_maps(cfg, prep, x, batch, ws, bs):
    import ml_dtypes
    bf16 = ml_dtypes.bfloat16
    x16 = np.ascontiguousarray(np.asarray(x, np.float32)).astype(bf16)
    batch = np.asarray(batch).astype(np.int64)
    wmats = np.stack([np.asarray(w, np.float32) for w in ws]).astype(bf16)
    biasb = np.stack([np.asarray(b, np.float32).reshape(1, cfg.d)
                      for b in bs]).astype(bf16)
    iota = np.tile(np.arange(cfg.blk, dtype=np.float32), (128, 1)).astype(bf16)

    # pooling one-hot: local node l (block l//blk, slot l%blk) -> graph id
    poolm = np.zeros((cfg.n_cores, cfg.blk, cfg.n_blk * cfg.n_graphs),
                     np.float32)
    c_idx = np.repeat(np.arange(cfg.n_cores), cfg.shard)
    l = np.tile(np.arange(cfg.shard), cfg.n_cores)
    poolm[c_idx, l % cfg.blk, (l // cfg.blk) * cfg.n_graphs + batch] = 1.0
    poolm16 = poolm.astype(bf16)

    in_maps = []
    for c in range(cfg.n_cores):
        in_maps.append({
            "t0": x16,
            "idxw": prep["idxw"][c],
            "slotw": prep["slotw"][c],
            "eww": prep["eww"][c],
            "poolm": poolm16[c],
            "iota": iota,
            "wmats": wmats,
            "biasb": biasb,
        })
    counts = np.bincount(batch, minlength=cfg.n_graphs).astype(np.float32)
    return in_maps, counts


_PROGRAM_CACHE = {}


def run(cfg, x, edge_index, edge_weight, batch, ws, bs, trace=False, trunc=""):
    prep = preprocess(cfg, edge_index, edge_weight)
    key = (cfg.n_nodes, cfg.n_cores, cfg.d, cfg.n_graphs, cfg.split, cfg.blk,
           prep["ep"], tuple(prep["seg"].ravel()), trunc)
    nc = _PROGRAM_CACHE.get(key)
    if nc is None:
        nc = build_program(cfg, prep["seg"], prep["seg_off"], prep["ep"],
                           trunc=trunc)
        _PROGRAM_CACHE[key] = nc
    in_maps, counts = make_in_maps(cfg, prep, x, batch, ws, bs)
    res = bass_utils.run_bass_kernel_spmd(
        nc, in_maps, core_ids=list(range(cfg.n_cores)), trace=trace)
    if trunc:
        return np.zeros((cfg.n_graphs, cfg.d), np.float32), res
    partial = np.zeros((cfg.n_graphs, cfg.d), np.float64)
    for c in range(cfg.n_cores):
        partial += res.results[c]["pool_out"].astype(np.float64)
    out = (partial / np.maximum(counts, 1.0)[:, None]).astype(np.float32)
    return out, res


def kernel(x, edge_index, edge_weight, batch, W0, b0, W1, b1, W2, b2):
    cfg = Cfg()
    trace = bool(int(os.environ.get("GCN_TRACE", "0")))
    out, _ = run(cfg, x, edge_index, edge_weight, batch,
                 [W0, W1, W2], [b0, b1, b2], trace=trace)
    return out


# revision 28
# speedup vs baseline: 1.2212x; 1.2212x over previous
"""Trainium2 Bass kernel for a 3-layer GCN encoder (PyG GCNConv x3 + global mean pool).

Strategy (8 NeuronCores):
  - Nodes sharded contiguously across cores (6250 each); edges (+self-loops)
    partitioned by destination, bucketed per 128-node dst block and split by
    source half (int16 gather-index limit), chunk-128 padded, sizes uniform
    across cores (SPMD program).
  - Symmetric normalization is folded on the host: ew'' = ew * dinv[src] *
    dinv[dst] (self-loop weight dinv[n]^2), so the device computes per layer
        agg[dst] = sum_e ew''_e * h[src_e]          (gathered bf16 rows)
        h'       = relu(agg @ W + b)
    with NO degree phase and NO per-layer dinv multiplies. The layer-0 table
    is x itself (bf16, replicated to every core's HBM by the host).
  - Gathers: dma_gather of 256B bf16 rows over 4 SWDGE queues. The
    descriptor ring is ucode-fixed at ~1024+16 entries per queue (larger
    dynamic_dma_scratch_size does NOT raise it), so calls are capped at
    1024 idxs. Pad positions use idx=0 with ew=0 (valid descriptor, zero
    contribution).
  - The weighted one-hot is built in TWO big-tile DVE tensor_tensor ops per
    (block,half) segment ((iota==slot), then *ew) with stride-0 broadcast
    APs — ~34x fewer DVE instructions than per-chunk tensor_scalar. The
    TensorEngine contracts gathered-rows x one-hot into PSUM (bf16 operands,
    f32 accumulate), seeds the bias with a K=1 ones x b matmul, applies W.
    Relu + PSUM->SBUF copies run on the otherwise-idle scalar engine.
  - h' (bf16) is written to the core's table shard and AllGathered between
    layers (bf16 halves collective bytes). Final global mean pool: per-block
    one-hot matmul accumulated in PSUM across all blocks; host sums the 8
    [64,128] partials and divides by graph counts.

Measured on 8xTRN2 (trace core 0): 2.11 ms vs 5.64 ms for the f32
per-chunk-one-hot baseline; rel err 2.3e-3 (gate 2e-2).
Notes from tuning: 64-node dst blocks halve DVE/PE active time but REGRESS
wall time (~3.1 ms) — 2x the (block,half) iterations doubles cross-engine
semaphore traffic and the gather pipeline stalls; 1920/3968-idx gather calls
crash (ring cap); negative pad idxs require num_idxs_reg == #valid and
uniform tails; dropping even the lightest 10% of edges exceeds the error
budget.
"""

import os
import sys

import numpy as np

for _p in ("/opt/trn_rl_repo",):
    if _p not in sys.path and os.path.isdir(_p):
        sys.path.insert(0, _p)

import concourse.bass as bass
import concourse.bacc as bacc
import concourse.tile as tile
import concourse.mybir as mybir
from concourse import bass_utils
from concourse.alu_op_type import AluOpType

F32 = mybir.dt.float32
BF16 = mybir.dt.bfloat16
I16 = mybir.dt.int16
AF = mybir.ActivationFunctionType

N_QUEUES = int(os.environ.get("GCN_QUEUES", "4"))
CALL_CHUNKS = int(os.environ.get("GCN_CALLCH", "8"))
NLOAD = 4  # const-load column slices (early gathers overlap the tail loads)
A_BLK = 25  # shard blocks in table-half A (rows 0:3200); rest in half B


class Cfg:
    def __init__(self, n_nodes=50000, n_cores=8, d=128, n_graphs=64,
                 split=32768, blk=128):
        assert n_nodes % n_cores == 0
        self.n_nodes = n_nodes
        self.n_cores = n_cores
        self.d = d
        self.n_graphs = n_graphs
        self.split = split  # gather-table split point (int16 index limit)
        self.shard = n_nodes // n_cores
        self.blk = blk
        self.n_blk = (self.shard + blk - 1) // blk

    def slots(self, b):
        return min(self.blk, self.shard - b * self.blk)


def preprocess(cfg, edge_index, edge_weight):
    """Host-side: degrees/dinv, normalization folding, edge bucketing.
    Returns per-core gather/one-hot arrays + core-uniform segment sizes."""
    src = np.asarray(edge_index)[0].astype(np.int64)
    dst = np.asarray(edge_index)[1].astype(np.int64)
    ew = np.asarray(edge_weight).astype(np.float64)
    n = cfg.n_nodes
    loop = np.arange(n, dtype=np.int64)
    src = np.concatenate([src, loop])
    dst = np.concatenate([dst, loop])
    ew = np.concatenate([ew, np.ones(n, np.float64)])
    ne = src.shape[0]

    deg = np.bincount(dst, weights=ew, minlength=n)  # >= 1 (self-loop)
    dinv = 1.0 / np.sqrt(deg)
    ew = (ew * dinv[src] * dinv[dst]).astype(np.float32)

    core = dst // cfg.shard
    loc = dst - core * cfg.shard
    blkid = loc // cfg.blk
    slot = (loc - blkid * cfg.blk).astype(np.float32)
    # gather-table halves: A = each core's shard rows [0, A_BLK*128),
    # B = the rest. Both halves have < 32768 rows (int16 idx) and half A
    # can be AllGathered mid-layer, as soon as its blocks are written.
    arows = A_BLK * 128
    brows = cfg.shard - arows
    csrc = src // cfg.shard
    srow = src - csrc * cfg.shard
    half = (srow >= arows).astype(np.int64)
    idxv = np.where(half == 1, csrc * brows + (srow - arows),
                    csrc * arows + srow)
    key = (core * cfg.n_blk + blkid) * 2 + half

    order = np.argsort(key, kind="stable")
    nkeys = cfg.n_cores * cfg.n_blk * 2
    cnt = np.bincount(key, minlength=nkeys).reshape(cfg.n_cores, cfg.n_blk * 2)
    # per-(block,half) chunk-padded sizes, shared across cores (SPMD program)
    seg = ((cnt.max(axis=0) + 127) // 128) * 128  # [n_blk*2]
    seg_off = np.concatenate([[0], np.cumsum(seg)])  # [n_blk*2+1]
    ep = int(seg_off[-1])

    # position of each edge inside its padded per-core segment
    cnt_flat = np.bincount(key, minlength=nkeys)
    starts = np.concatenate([[0], np.cumsum(cnt_flat)])[:-1]
    sk = key[order]
    rank = np.arange(ne) - starts[sk]
    bh = sk % (cfg.n_blk * 2)
    pos = seg_off[bh] + rank
    core_s = sk // (cfg.n_blk * 2)

    # pads: idx=0 with slot/ew = 0 (valid descriptor, zero contribution)
    idx16 = np.zeros((cfg.n_cores, ep), np.int16)
    slotf = np.zeros((cfg.n_cores, ep), np.float32)
    ewf = np.zeros((cfg.n_cores, ep), np.float32)
    idx16[core_s, pos] = idxv[order].astype(np.int16)
    slotf[core_s, pos] = slot[order]
    ewf[core_s, pos] = ew[order]

    # wrapped index layout: edge i -> [i%16, i//16], replicated to 128 partitions
    idxw = idx16.reshape(cfg.n_cores, ep // 16, 16).transpose(0, 2, 1)
    idxw = np.tile(idxw, (1, 8, 1)).copy()  # [cores, 128, ep//16]
    # chunk layout for the one-hot build: edge i -> [i%128, i//128], bf16
    import ml_dtypes
    bf16 = ml_dtypes.bfloat16
    slotw = np.ascontiguousarray(
        slotf.reshape(cfg.n_cores, ep // 128, 128).transpose(0, 2, 1)
    ).astype(bf16)
    eww = np.ascontiguousarray(
        ewf.reshape(cfg.n_cores, ep // 128, 128).transpose(0, 2, 1)
    ).astype(bf16)

    return dict(seg=seg.reshape(cfg.n_blk, 2), seg_off=seg_off, ep=ep,
                idxw=idxw, slotw=slotw, eww=eww)


def build_program(cfg, seg, seg_off, ep, trunc=""):
    """Build the SPMD Bass/Tile program. Trip counts depend only on seg/ep.

    trunc: debug knob - "G" gathers only, "OH" +one-hot, "L0"/"L1" stop
    after layer 0/1 (outputs garbage)."""
    n_blk, d, g64 = cfg.n_blk, cfg.d, cfg.n_graphs
    nc = bacc.Bacc("TRN2", target_bir_lowering=False, debug=False,
                   enable_asserts=False, num_devices=cfg.n_cores,
                   num_swdge_queues=N_QUEUES)

    arows = A_BLK * 128
    brows = cfg.shard - arows
    t0a_in = nc.dram_tensor("t0a", [cfg.n_cores * arows, d], BF16,
                            kind="ExternalInput")
    t0b_in = nc.dram_tensor("t0b", [cfg.n_cores * brows, d], BF16,
                            kind="ExternalInput")
    idx_in = nc.dram_tensor("idxw", [128, ep // 16], I16, kind="ExternalInput")
    slot_in = nc.dram_tensor("slotw", [128, ep // 128], BF16, kind="ExternalInput")
    ew_in = nc.dram_tensor("eww", [128, ep // 128], BF16, kind="ExternalInput")
    pool_in = nc.dram_tensor("poolm", [128, n_blk * g64], BF16, kind="ExternalInput")
    iota_in = nc.dram_tensor("iota", [128, 128], BF16, kind="ExternalInput")
    w_in = nc.dram_tensor("wmats", [3, d, d], BF16, kind="ExternalInput")
    b_in = nc.dram_tensor("biasb", [3, 1, d], BF16, kind="ExternalInput")
    out_t = nc.dram_tensor("pool_out", [g64, d], F32, kind="ExternalOutput")

    g_loc_a = [None] + [nc.dram_tensor(f"g_loca{k}", [arows, d], BF16,
                                       kind="Internal") for k in (1, 2)]
    g_loc_b = [None] + [nc.dram_tensor(f"g_locb{k}", [brows, d], BF16,
                                       kind="Internal") for k in (1, 2)]
    g_full_a = [None] + [nc.dram_tensor(f"g_fulla{k}",
                                        [cfg.n_cores * arows, d], BF16,
                                        kind="Internal", addr_space="Shared")
                         for k in (1, 2)]
    g_full_b = [None] + [nc.dram_tensor(f"g_fullb{k}",
                                        [cfg.n_cores * brows, d], BF16,
                                        kind="Internal", addr_space="Shared")
                         for k in (1, 2)]
    rg = [list(range(cfg.n_cores))]

    n_layers = {"G": 1, "OH": 1, "L0": 1, "L1": 2}.get(trunc, 3)
    stop_at = {"G": 1, "OH": 2}.get(trunc, 99)

    with tile.TileContext(nc) as tc:
        with tc.tile_pool(name="const", bufs=1) as cp:
            iota_sb = cp.tile([128, 128], BF16, tag="iota")
            nc.sync.dma_start(iota_sb[:, :], iota_in.ap())
            ones_sb = cp.tile([1, 128], BF16, tag="ones")
            nc.vector.memset(ones_sb[:, :], 1.0)
            w_sb = []
            b_sb = []
            for k in range(3):
                wt = cp.tile([d, d], BF16, tag=f"w{k}", name=f"w{k}")
                nc.sync.dma_start(wt[:, :], w_in.ap()[k, :, :])
                w_sb.append(wt)
                bt = cp.tile([1, d], BF16, tag=f"b{k}", name=f"b{k}")
                nc.sync.dma_start(bt[:, :], b_in.ap()[k, :, :])
                b_sb.append(bt)
            poolm_sb = cp.tile([128, n_blk * g64], BF16, tag="poolm")
            nc.sync.dma_start(poolm_sb[:, :], pool_in.ap())
            # sliced const loads so early gathers don't wait on one huge DMA
            idx_sb = cp.tile([128, ep // 16], I16, tag="idx")
            slot_sb = cp.tile([128, ep // 128], BF16, tag="slot")
            ew_sb = cp.tile([128, ep // 128], BF16, tag="ew")
            w16 = ep // 16 // NLOAD
            w128 = ep // 128 // NLOAD
            for i in range(NLOAD):
                a16 = i * w16
                b16 = (i + 1) * w16 if i < NLOAD - 1 else ep // 16
                a12 = i * w128
                b12 = (i + 1) * w128 if i < NLOAD - 1 else ep // 128
                nc.sync.dma_start(idx_sb[:, a16:b16], idx_in.ap()[:, a16:b16])
                nc.sync.dma_start(slot_sb[:, a12:b12], slot_in.ap()[:, a12:b12])
                nc.sync.dma_start(ew_sb[:, a12:b12], ew_in.ap()[:, a12:b12])

            with tc.tile_pool(name="aggp", bufs=2, space="PSUM") as psA, \
                 tc.tile_pool(name="outp", bufs=2, space="PSUM") as psB, \
                 tc.tile_pool(name="poolp", bufs=1, space="PSUM") as psC, \
                 tc.tile_pool(name="ohp", bufs=4) as ohp, \
                 tc.tile_pool(name="stage", bufs=3) as stp, \
                 tc.tile_pool(name="workp", bufs=3) as wp:
                pool_acc = psC.tile([g64, d], F32, tag="pacc")
                gq = [0]  # round-robin gather queue counter
                for k in range(n_layers):
                    bases = ((t0a_in.ap(), t0b_in.ap()) if k == 0 else
                             (g_full_a[k].ap(), g_full_b[k].ap()))
                    for b in range(n_blk):
                        s = cfg.slots(b)
                        stg = {}
                        ohg = {}
                        for h in (0, 1):
                            n_ch = int(seg[b, h]) // 128
                            if n_ch == 0:
                                continue
                            c0seg = int(seg_off[b * 2 + h]) // 128
                            o16 = int(seg_off[b * 2 + h]) // 16
                            st = stp.tile([128, n_ch, d], BF16, tag=f"st{h}",
                                          name=f"st{k}_{b}_{h}")
                            base = bases[h]
                            for c0 in range(0, n_ch, CALL_CHUNKS):
                                c1 = min(c0 + CALL_CHUNKS, n_ch)
                                nidx = (c1 - c0) * 128
                                so16 = o16 + c0 * 8
                                nc.gpsimd.dma_gather(
                                    st[:, c0:c1, :], base,
                                    idx_sb[:, so16:so16 + nidx // 16],
                                    nidx, nidx, d,
                                    queue_num=gq[0] % N_QUEUES)
                                gq[0] += 1
                            stg[h] = st
                            if stop_at < 2:
                                continue
                            # weighted one-hot, two big-tile DVE passes
                            oh = ohp.tile([128, n_ch, 128], BF16, tag=f"oh{h}",
                                          name=f"oh{k}_{b}_{h}")
                            iota_b = (iota_sb[:, :].unsqueeze(1)
                                      .broadcast_to([128, n_ch, 128]))
                            slot_b = (slot_sb[:, c0seg:c0seg + n_ch]
                                      .unsqueeze(2)
                                      .broadcast_to([128, n_ch, 128]))
                            ew_b = (ew_sb[:, c0seg:c0seg + n_ch]
                                    .unsqueeze(2)
                                    .broadcast_to([128, n_ch, 128]))
                            nc.vector.tensor_tensor(
                                oh[:, :, :], iota_b, slot_b,
                                AluOpType.is_equal)
                            nc.vector.tensor_tensor(
                                oh[:, :, :], oh[:, :, :], ew_b,
                                AluOpType.mult)
                            ohg[h] = oh
                        if stop_at < 3:
                            continue
                        pagg = psA.tile([d, 128], F32, tag="agg",
                                        name=f"agg{k}_{b}")
                        cols = [(h, i) for h in (0, 1)
                                for i in range(int(seg[b, h]) // 128)]
                        for j, (h, i) in enumerate(cols):
                            nc.tensor.matmul(pagg[:, :s], stg[h][:, i, :],
                                             ohg[h][:, i, :s],
                                             start=(j == 0),
                                             stop=(j == len(cols) - 1))
                        aggT = wp.tile([d, 128], BF16, tag="aggT",
                                       name=f"aggT{k}_{b}")
                        nc.scalar.copy(aggT[:, :s], pagg[:, :s])
                        pout = psB.tile([128, d], F32, tag="out",
                                        name=f"out{k}_{b}")
                        nc.tensor.matmul(pout[:s, :], ones_sb[:, :s],
                                         b_sb[k][:, :], start=True, stop=False)
                        nc.tensor.matmul(pout[:s, :], aggT[:, :s], w_sb[k][:, :],
                                         start=False, stop=True)
                        if k < 2:
                            ht = wp.tile([128, d], BF16, tag="ht",
                                         name=f"ht{k}_{b}")
                            nc.scalar.activation(ht[:s, :], pout[:s, :], AF.Relu)
                            if b < A_BLK:
                                dst_ap = g_loc_a[k + 1].ap()[
                                    b * 128:b * 128 + s, :]
                            else:
                                r0 = (b - A_BLK) * 128
                                dst_ap = g_loc_b[k + 1].ap()[r0:r0 + s, :]
                            nc.sync.dma_start(dst_ap, ht[:s, :])
                        else:
                            t2 = wp.tile([128, d], BF16, tag="t2",
                                         name=f"t2_{b}")
                            nc.scalar.copy(t2[:s, :], pout[:s, :])
                            nc.tensor.matmul(
                                pool_acc[:, :],
                                poolm_sb[:s, b * g64:(b + 1) * g64],
                                t2[:s, :], start=(b == 0),
                                stop=(b == n_blk - 1))
                        # half A of h' is complete two blocks after A_BLK:
                        # AllGather it mid-layer, hidden behind the remaining
                        # blocks' compute, leaving only half B's (smaller)
                        # collective on the layer boundary.
                        if k < 2 and stop_at >= 3 and b == A_BLK + 1:
                            nc.gpsimd.collective_compute(
                                "AllGather", AluOpType.bypass,
                                replica_groups=rg,
                                ins=[g_loc_a[k + 1].ap()],
                                outs=[g_full_a[k + 1].ap()])
                    if k < 2 and stop_at >= 3:
                        nc.gpsimd.collective_compute(
                            "AllGather", AluOpType.bypass, replica_groups=rg,
                            ins=[g_loc_b[k + 1].ap()], outs=[g_full_b[k + 1].ap()])

                if n_layers == 3 and stop_at >= 3:
                    pf = wp.tile([g64, d], F32, tag="pf", name="poolf")
                    nc.scalar.copy(pf[:, :], pool_acc[:, :])
                    nc.sync.dma_start(out_t.ap(), pf[:, :])

    nc.compile()
    return nc


def make_in_maps(cfg, prep, x, batch, ws, bs):
    import ml_dtypes
    bf16 = ml_dtypes.bfloat16
    x16 = np.ascontiguousarray(np.asarray(x, np.float32)).astype(bf16)
    arows = A_BLK * 128
    xr = x16.reshape(cfg.n_cores, cfg.shard, cfg.d)
    t0a = np.ascontiguousarray(xr[:, :arows]).reshape(-1, cfg.d)
    t0b = np.ascontiguousarray(xr[:, arows:]).reshape(-1, cfg.d)
    batch = np.asarray(batch).astype(np.int64)
    wmats = np.stack([np.asarray(w, np.float32) for w in ws]).astype(bf16)
    biasb = np.stack([np.asarray(b, np.float32).reshape(1, cfg.d)
                      for b in bs]).astype(bf16)
    iota = np.tile(np.arange(128, dtype=np.float32), (128, 1)).astype(bf16)

    # pooling one-hot: local node l (block b=l//128, part p=l%128) -> graph id
    poolm = np.zeros((cfg.n_cores, 128, cfg.n_blk * cfg.n_graphs), np.float32)
    c_idx = np.repeat(np.arange(cfg.n_cores), cfg.shard)
    l = np.tile(np.arange(cfg.shard), cfg.n_cores)
    poolm[c_idx, l % 128, (l // 128) * cfg.n_graphs + batch] = 1.0
    poolm16 = poolm.astype(bf16)

    in_maps = []
    for c in range(cfg.n_cores):
        in_maps.append({
            "t0a": t0a,
            "t0b": t0b,
            "idxw": prep["idxw"][c],
            "slotw": prep["slotw"][c],
            "eww": prep["eww"][c],
            "poolm": poolm16[c],
            "iota": iota,
            "wmats": wmats,
            "biasb": biasb,
        })
    counts = np.bincount(batch, minlength=cfg.n_graphs).astype(np.float32)
    return in_maps, counts


_PROGRAM_CACHE = {}


def run(cfg, x, edge_index, edge_weight, batch, ws, bs, trace=False, trunc=""):
    prep = preprocess(cfg, edge_index, edge_weight)
    key = (cfg.n_nodes, cfg.n_cores, cfg.d, cfg.n_graphs, cfg.split, cfg.blk,
           prep["ep"], tuple(prep["seg"].ravel()), trunc)
    nc = _PROGRAM_CACHE.get(key)
    if nc is None:
        nc = build_program(cfg, prep["seg"], prep["seg_off"], prep["ep"],
                           trunc=trunc)
        _PROGRAM_CACHE[key] = nc
    in_maps, counts = make_in_maps(cfg, prep, x, batch, ws, bs)
    res = bass_utils.run_bass_kernel_spmd(
        nc, in_maps, core_ids=list(range(cfg.n_cores)), trace=trace)
    if trunc:
        return np.zeros((cfg.n_graphs, cfg.d), np.float32), res
    partial = np.zeros((cfg.n_graphs, cfg.d), np.float64)
    for c in range(cfg.n_cores):
        partial += res.results[c]["pool_out"].astype(np.float64)
    out = (partial / np.maximum(counts, 1.0)[:, None]).astype(np.float32)
    return out, res


def kernel(x, edge_index, edge_weight, batch, W0, b0, W1, b1, W2, b2):
    cfg = Cfg()
    trace = bool(int(os.environ.get("GCN_TRACE", "0")))
    out, _ = run(cfg, x, edge_index, edge_weight, batch,
                 [W0, W1, W2], [b0, b1, b2], trace=trace)
    return out
